# revision 1
# baseline (speedup 1.0000x reference)
"""Trainium2 Bass kernel for nn_KernelGraphAttentionNetwork.

Strategy (8 NeuronCores):
  - Shard: batch (2) x S1-quarters (4)  -> 8 shards. Each core computes the
    edge-kernel for its 4 query sentences i against all 16 key sentences j:
      sim = rhat_i @ rhat_all^T          (PE, fp32, contraction over D=768)
      rbf_k = exp(-(sim-mu_k)^2/(2 s_k^2))  (ScalarE: Square-act + Exp-act)
      pool  = sum_q rbf_k                (VectorE reduce over T2 within j)
      Ke    = ln(clip(pool, 1e-6))       (ScalarE Ln)
      logit = sum_k Ke * w_sel[k]        (VectorE mul + reduce)
    and returns logits (4 x 64 x 16 per core, 16KB).
  - Host: pre-normalizes + pre-transposes reps (so cosine sim is a pure
    matmul and both operands are D-major), then finishes the tiny coupled
    tail: T1-softmax, z_hat einsum, gating MLP, beta softmax over S1 (the
    "small all-gather" of the sharding hint is the host gather), label
    head, node kernel, rationale softmax.

Layout on device (per core):
  partition = (2 local query sentences x 64 T1-tokens) = 128
  free      = (16 key sentences x 64 T2-tokens)        = 1024
  Two such tiles (ip = 0,1) cover the core's 4 query sentences.
"""

import numpy as np

KERNEL = 11
B, S, T, D = 2, 16, 64, 768
EPS = 1e-6
CLAMP_MIN = 1e-6
N_CORES = 8


def _kernel_mus(n):
    mus = [1.0]
    if n == 1:
        return mus
    b = 2.0 / (n - 1)
    mus.append(1.0 - b / 2.0)
    for i in range(1, n - 1):
        mus.append(mus[i] - b)
    return mus


MU = np.asarray(_kernel_mus(KERNEL), dtype=np.float64)
SIGMA = np.asarray([0.001] + [0.1] * (KERNEL - 1), dtype=np.float64)

_NC_CACHE = {}
LAST_RESULTS = None


def _build_nc():
    """Build the Bass module (same NEFF for every core; per-core data differs)."""
    import concourse.bass as bass
    import concourse.tile as tile
    from concourse import bacc, mybir

    nc = bacc.Bacc(
        "TRN2",
        target_bir_lowering=False,
        debug=False,
        enable_asserts=False,
    )
    f32 = mybir.dt.float32
    AF = mybir.ActivationFunctionType
    NK = KERNEL - 1  # k=0 (exact-match, sigma=1e-3) is constant over T1 -> softmax-invariant

    bf16 = mybir.dt.bfloat16
    rhat_t = nc.dram_tensor("rhat_t", (D, S * T), bf16, kind="ExternalInput").ap()
    rhat_i = nc.dram_tensor("rhat_i", (D, 256), bf16, kind="ExternalInput").ap()
    consts = nc.dram_tensor(
        "consts", (S * NK + NK,), f32, kind="ExternalInput"
    ).ap()
    logits_out = nc.dram_tensor(
        "logits_out", (2, 128, S), f32, kind="ExternalOutput"
    ).ap()

    with tile.TileContext(nc) as tc:
        with (
            tc.tile_pool(name="rt", bufs=1) as rt_pool,
            tc.tile_pool(name="ri", bufs=1) as ri_pool,
            tc.tile_pool(name="cst", bufs=1) as cst_pool,
            tc.tile_pool(name="psum", bufs=4, space="PSUM") as psum_pool,
            tc.tile_pool(name="work", bufs=4) as work_pool,
            tc.tile_pool(name="pacc", bufs=2) as pacc_pool,
            tc.tile_pool(name="outs", bufs=2) as out_pool,
        ):
            # --- load inputs ---
            rt = []
            ri = []
            for dc in range(6):
                t_ = rt_pool.tile([128, S * T], bf16, tag=f"rt{dc}")
                nc.sync.dma_start(out=t_, in_=rhat_t[dc * 128 : (dc + 1) * 128, :])
                rt.append(t_)
                t2 = ri_pool.tile([128, 256], bf16, tag=f"ri{dc}")
                nc.sync.dma_start(out=t2, in_=rhat_i[dc * 128 : (dc + 1) * 128, :])
                ri.append(t2)
            # broadcast w_sel-per-(j,k) to all 128 partitions
            wsel_b = cst_pool.tile([128, S * NK], f32)
            bcast = bass.AP(
                tensor=consts.tensor,
                offset=consts.offset,
                ap=[[0, 128], [1, S * NK]],
            )
            nc.sync.dma_start(out=wsel_b, in_=bcast)
            # broadcast -mu[k] per partition for Square-act bias
            negmu_b = cst_pool.tile([128, NK], f32)
            bcast2 = bass.AP(
                tensor=consts.tensor,
                offset=consts.offset + S * NK,
                ap=[[0, 128], [1, NK]],
            )
            nc.sync.dma_start(out=negmu_b, in_=bcast2)

            for ip in range(2):
                # --- sim matmul: PSUM (128, 512) x 2 ---
                sim_ps = []
                for nch in range(2):
                    ps = psum_pool.tile([128, 512], f32, tag=f"sim{nch}")
                    for dc in range(6):
                        nc.tensor.matmul(
                            ps,
                            lhsT=ri[dc][:, ip * 128 : (ip + 1) * 128],
                            rhs=rt[dc][:, nch * 512 : (nch + 1) * 512],
                            start=(dc == 0),
                            stop=(dc == 5),
                        )
                    sim_ps.append(ps)

                # --- RBF + pool over q ---
                poolk = pacc_pool.tile([128, S, NK], f32)
                for kk in range(NK):
                    k = kk + 1
                    alpha = float(0.5 / (SIGMA[k] ** 2))
                    d2 = work_pool.tile([128, 1024], f32, tag="d2")
                    for nch in range(2):
                        nc.scalar.activation(
                            out=d2[:, nch * 512 : (nch + 1) * 512],
                            in_=sim_ps[nch],
                            func=AF.Square,
                            bias=negmu_b[:, kk : kk + 1],
                            scale=1.0,
                        )
                    e = work_pool.tile([128, 1024], f32, tag="e")
                    nc.scalar.activation(out=e, in_=d2, func=AF.Exp, scale=-alpha)
                    nc.vector.reduce_sum(
                        out=poolk[:, :, kk : kk + 1],
                        in_=e.rearrange("p (j q) -> p j q", q=T),
                        axis=mybir.AxisListType.X,
                    )

                # --- Ke = ln(clip(pool)), logits = sum_k Ke*w ---
                pkf = poolk.rearrange("p j k -> p (j k)")
                nc.vector.tensor_scalar_max(out=pkf, in0=pkf, scalar1=CLAMP_MIN)
                ke = work_pool.tile([128, S * NK], f32, tag="ke")
                nc.scalar.activation(out=ke, in_=pkf, func=AF.Ln)
                nc.vector.tensor_mul(out=ke, in0=ke, in1=wsel_b)
                lg = out_pool.tile([128, S], f32, tag="lg")
                nc.vector.reduce_sum(
                    out=lg,
                    in_=ke.rearrange("p (j k) -> p j k", k=KERNEL - 1),
                    axis=mybir.AxisListType.X,
                )
                nc.sync.dma_start(out=logits_out[ip], in_=lg)
    nc.finalize()
    return nc


def _reference_numpy(claim_reps, sentence_token_reps, claim_token_mask, token_mask,
                     w_sel, b_sel, w_g1, b_g1, w_g2, b_g2, w_rat, b_rat,
                     w_lab, b_lab):
    """Pure-numpy fallback (only used if masks are not all-ones)."""
    reps = sentence_token_reps.astype(np.float64)
    maskf = token_mask.astype(np.float64)
    b_, s_, t_, d_ = reps.shape
    norms = np.linalg.norm(reps, axis=-1)
    dot = np.einsum("bipd,bjqd->bijpq", reps, reps)
    sim = dot / np.maximum(norms[:, :, None, :, None] * norms[:, None, :, None, :], EPS)
    rbf = np.exp(-0.5 * ((sim[..., None] - MU) / SIGMA) ** 2)
    pool = rbf.sum(axis=4) * maskf[:, None, :, :, None]
    Ke = np.log(np.clip(pool, CLAMP_MIN, None))
    logits = Ke @ w_sel + b_sel
    m2 = np.broadcast_to(token_mask[:, None, :, :, None], logits.shape)
    lg = np.where(m2, logits, -10000.0)
    return _finish(reps, norms, lg[..., 0], claim_reps, token_mask,
                   w_g1, b_g1, w_g2, b_g2, w_rat, b_rat, w_lab, b_lab)


def _softmax(x, axis):
    m = np.max(x, axis=axis, keepdims=True)
    e = np.exp(x - m)
    return e / e.sum(axis=axis, keepdims=True)


def _finish(reps, norms, logits, claim_reps, token_mask,
            w_g1, b_g1, w_g2, b_g2, w_rat, b_rat, w_lab, b_lab):
    """Shared tail: logits (B,S1,S2,T1) -> output (B,3). float64 numpy."""
    t_ = reps.shape[2]
    attn = _softmax(logits, axis=3)  # (B,S1,S2,T1) softmax over T1
    z_hat = np.einsum("bjtd,bijt->bijd", reps, attn)
    z = reps[:, :, 0, :]
    z_exp = np.broadcast_to(z[:, None, :, :], z_hat.shape)
    hcat = np.concatenate([z_exp, z_hat], axis=-1)
    h = np.maximum(hcat @ w_g1 + b_g1, 0.0)
    beta = _softmax(h @ w_g2 + b_g2, axis=1)
    v = np.concatenate([np.sum(beta * z_hat, axis=1), z], axis=-1)
    slp = _softmax(v @ w_lab + b_lab, axis=-1)

    ncl = np.linalg.norm(claim_reps, axis=-1)
    dotn = np.einsum("btd,bstd->bst", claim_reps, reps)
    simn = dotn / np.maximum(ncl[:, None, :] * norms, EPS)
    rbfn = np.exp(-0.5 * ((simn[..., None] - MU) / SIGMA) ** 2)
    pooln = rbfn * token_mask.astype(np.float64)[..., None] * float(t_)
    phi = np.mean(np.log(np.clip(pooln, CLAMP_MIN, None)), axis=-2)
    rationale = _softmax(phi @ w_rat + b_rat, axis=1)
    return np.sum(slp * rationale, axis=1)


def kernel(**inputs):
    global LAST_RESULTS
    claim_reps = np.asarray(inputs["claim_reps"], dtype=np.float32)
    reps = np.asarray(inputs["sentence_token_reps"], dtype=np.float32)
    claim_token_mask = np.asarray(inputs["claim_token_mask"])
    token_mask = np.asarray(inputs["token_mask"])
    w_sel = np.asarray(inputs["w_sel"], dtype=np.float32)
    b_sel = np.asarray(inputs["b_sel"], dtype=np.float32)
    w_g1 = np.asarray(inputs["w_g1"], dtype=np.float32)
    b_g1 = np.asarray(inputs["b_g1"], dtype=np.float32)
    w_g2 = np.asarray(inputs["w_g2"], dtype=np.float32)
    b_g2 = np.asarray(inputs["b_g2"], dtype=np.float32)
    w_rat = np.asarray(inputs["w_rat"], dtype=np.float32)
    b_rat = np.asarray(inputs["b_rat"], dtype=np.float32)
    w_lab = np.asarray(inputs["w_lab"], dtype=np.float32)
    b_lab = np.asarray(inputs["b_lab"], dtype=np.float32)

    if not (token_mask.all() and claim_token_mask.all()):
        out = _reference_numpy(claim_reps, reps, claim_token_mask, token_mask,
                               w_sel, b_sel, w_g1, b_g1, w_g2, b_g2,
                               w_rat, b_rat, w_lab, b_lab)
        return out.astype(np.float32)

    from concourse.bass_utils import run_bass_kernel_spmd

    # --- host prep: normalize + transpose ---
    norms = np.linalg.norm(reps, axis=-1)  # (B,S,T)
    rhat = reps / norms[..., None]
    import ml_dtypes
    rhat_t = [
        np.ascontiguousarray(rhat[b].reshape(S * T, D).T).astype(ml_dtypes.bfloat16)
        for b in range(B)
    ]

    wk = np.concatenate(
        [np.tile(w_sel[1:, 0].astype(np.float32), S), (-MU[1:]).astype(np.float32)]
    ).astype(np.float32)  # (S*NK + NK,)

    in_maps = []
    for c in range(N_CORES):
        b, ig = divmod(c, 4)
        in_maps.append(
            {
                "rhat_t": rhat_t[b],
                "rhat_i": np.ascontiguousarray(rhat_t[b][:, ig * 256 : (ig + 1) * 256]),
                "consts": wk,
            }
        )

    key = "nc"
    if key not in _NC_CACHE:
        _NC_CACHE[key] = _build_nc()
    nc = _NC_CACHE[key]

    res = run_bass_kernel_spmd(nc, in_maps, core_ids=list(range(N_CORES)))
    LAST_RESULTS = res

    # --- gather: logits_out per core (2, 128, 16) -> (B, S1, S2, T1) ---
    logits = np.empty((B, S, S, T), dtype=np.float32)
    for c in range(N_CORES):
        b, ig = divmod(c, 4)
        lo = res.results[c]["logits_out"]  # (2, 128, 16)
        for ip in range(2):
            for a in range(2):
                i = ig * 4 + ip * 2 + a
                # partition rows a*64..a*64+63 = T1 tokens; cols = j
                logits[b, i, :, :] = np.transpose(lo[ip, a * 64 : (a + 1) * 64, :])
    # add b_sel (constant over T1 — softmax-invariant, but keep exactness)
    logits64 = logits.astype(np.float64) + float(b_sel[0])

    out = _finish(reps.astype(np.float64), norms.astype(np.float64), logits64,
                  claim_reps.astype(np.float64), token_mask,
                  w_g1, b_g1, w_g2, b_g2, w_rat, b_rat, w_lab, b_lab)
    return out.astype(np.float32)



# revision 2
# speedup vs baseline: 28.4033x; 28.4033x over previous
"""Trainium2 Bass kernel for nn_KernelGraphAttentionNetwork.

Strategy (8 NeuronCores):
  Ship each core a UNIQUE (768, 256) bf16 slice of the normalized,
  transposed sentence reps (cores 0-3: batch 0 query-groups 0-3,
  cores 4-7: batch 1). On device, an AllGather over NeuronLink gives
  every core all 2048 columns (both batches); each core then computes
  the edge-kernel logits for its own 256 query tokens against all 32
  gathered sentences:
      sim   = rhat_own^T @ rhat_all        (PE, contraction over D=768)
      rbf_k = exp(-(sim-mu_k)^2/(2 s_k^2)) (ScalarE Square+Exp)
      pool  = sum_q rbf_k                  (VectorE reduce over T2)
      logit = sum_k ln(clip(pool)) * w_sel (ScalarE Ln + VectorE)
  Host keeps the 16 columns of the core's own batch and finishes the
  small coupled tail in f32 numpy: T1-softmax, z_hat einsum, gating
  MLP, beta softmax over S1, label head, node kernel, rationale.

  All heavy one-time setup (jax/concourse import, Bass build, NEFF
  compile, executable load, axon warm-up) happens at module import so
  kernel() itself is a single warm dispatch. The device kernel is
  partition-id free (uniform SPMD program; per-core behavior comes
  only from per-core input data), executed on cores 0-7 through the
  same bass_exec/PJRT path run_bass_kernel_spmd uses under axon.
"""

import numpy as np

KERNEL = 11
B, S, T, D = 2, 16, 64, 768
SG = 2 * S           # 32 gathered sentences (both batches)
EPS = 1e-6
CLAMP_MIN = 1e-6
N_CORES = 8
NK = KERNEL - 1      # k=0 (sigma=1e-3) is constant over T1 -> softmax-invariant


def _kernel_mus(n):
    mus = [1.0]
    if n == 1:
        return mus
    b = 2.0 / (n - 1)
    mus.append(1.0 - b / 2.0)
    for i in range(1, n - 1):
        mus.append(mus[i] - b)
    return mus


MU = np.asarray(_kernel_mus(KERNEL), dtype=np.float64)
SIGMA = np.asarray([0.001] + [0.1] * (KERNEL - 1), dtype=np.float64)
MU32 = MU.astype(np.float32)
SIG32 = SIGMA.astype(np.float32)

_STATE = {}
LAST_RESULTS = None


def _build_sharded():
    import jax
    from jax.sharding import Mesh, PartitionSpec as P
    from concourse.bass2jax import bass_jit, bass_shard_map
    from concourse import mybir
    import concourse.bass as bass
    import concourse.tile as tile

    @bass_jit(trn_type="TRN2", enable_asserts=False, num_devices=N_CORES)
    def _edge_logits_ag(nc, rpart, consts):
        f32 = mybir.dt.float32
        bf16 = mybir.dt.bfloat16
        AF = mybir.ActivationFunctionType
        logits_out = nc.dram_tensor(
            "logits_out", [2, 128, SG], f32, kind="ExternalOutput"
        )
        rpart = rpart[:]
        consts = consts[:]

        with tile.TileContext(nc) as tc:
            with (
                tc.tile_pool(name="dram", bufs=1, space="DRAM") as dram_pool,
                tc.tile_pool(name="rt", bufs=1) as rt_pool,
                tc.tile_pool(name="ri", bufs=1) as ri_pool,
                tc.tile_pool(name="cst", bufs=1) as cst_pool,
                tc.tile_pool(name="psum", bufs=2, space="PSUM") as psum_pool,
                tc.tile_pool(name="work", bufs=4) as work_pool,
                tc.tile_pool(name="pacc", bufs=2) as pacc_pool,
                tc.tile_pool(name="outs", bufs=2) as out_pool,
            ):
                # all-gather the unique slices via DRAM bounce buffers
                in_bounce = dram_pool.tile([D, 256], bf16, tag="inb")
                ag_out = dram_pool.tile([N_CORES * D, 256], bf16, tag="agb")
                nc.gpsimd.dma_start(in_bounce[:], rpart)
                nc.gpsimd.collective_compute(
                    "AllGather",
                    mybir.AluOpType.bypass,
                    replica_groups=[list(range(N_CORES))],
                    ins=[in_bounce.opt()],
                    outs=[ag_out.opt()],
                )

                ri = []  # own queries (lhsT): 6 x (128, 256)
                for dc in range(6):
                    t2 = ri_pool.tile([128, 256], bf16, tag=f"ri{dc}")
                    nc.sync.dma_start(out=t2, in_=rpart[dc * 128 : (dc + 1) * 128, :])
                    ri.append(t2)
                rt = []  # gathered keys: 6 x (128, 2048)
                agp = ag_out[:]
                for dc in range(6):
                    t_ = rt_pool.tile([128, SG * T], bf16, tag=f"rt{dc}")
                    for g in range(N_CORES):
                        nc.sync.dma_start(
                            out=t_[:, g * 256 : (g + 1) * 256],
                            in_=agp[g * D + dc * 128 : g * D + (dc + 1) * 128, :],
                        )
                    rt.append(t_)

                wsel_b = cst_pool.tile([128, SG * NK], f32)
                bcast = bass.AP(
                    tensor=consts.tensor,
                    offset=consts.offset,
                    ap=[[0, 128], [1, SG * NK]],
                )
                nc.sync.dma_start(out=wsel_b, in_=bcast)
                negmu_b = cst_pool.tile([128, NK], f32)
                bcast2 = bass.AP(
                    tensor=consts.tensor,
                    offset=consts.offset + SG * NK,
                    ap=[[0, 128], [1, NK]],
                )
                nc.sync.dma_start(out=negmu_b, in_=bcast2)

                NCH = (SG * T) // 512
                for ip in range(2):
                    sim_ps = []
                    for nch in range(NCH):
                        ps = psum_pool.tile([128, 512], f32, tag=f"sim{nch}")
                        for dc in range(6):
                            nc.tensor.matmul(
                                ps,
                                lhsT=ri[dc][:, ip * 128 : (ip + 1) * 128],
                                rhs=rt[dc][:, nch * 512 : (nch + 1) * 512],
                                start=(dc == 0),
                                stop=(dc == 5),
                            )
                        sim_ps.append(ps)

                    poolk = pacc_pool.tile([128, SG, NK], f32)
                    for kk in range(NK):
                        k = kk + 1
                        alpha = float(0.5 / (SIGMA[k] ** 2))
                        d2 = work_pool.tile([128, SG * T], f32, tag="d2")
                        for nch in range(NCH):
                            nc.scalar.activation(
                                out=d2[:, nch * 512 : (nch + 1) * 512],
                                in_=sim_ps[nch],
                                func=AF.Square,
                                bias=negmu_b[:, kk : kk + 1],
                                scale=1.0,
                            )
                        e = work_pool.tile([128, SG * T], f32, tag="e")
                        nc.scalar.activation(out=e, in_=d2, func=AF.Exp, scale=-alpha)
                        nc.vector.reduce_sum(
                            out=poolk[:, :, kk : kk + 1],
                            in_=e.rearrange("p (j q) -> p j q", q=T),
                            axis=mybir.AxisListType.X,
                        )

                    pkf = poolk.rearrange("p j k -> p (j k)")
                    nc.vector.tensor_scalar_max(out=pkf, in0=pkf, scalar1=CLAMP_MIN)
                    ke = work_pool.tile([128, SG * NK], f32, tag="ke")
                    nc.scalar.activation(out=ke, in_=pkf, func=AF.Ln)
                    nc.vector.tensor_mul(out=ke, in0=ke, in1=wsel_b)
                    lg = out_pool.tile([128, SG], f32, tag="lg")
                    nc.vector.reduce_sum(
                        out=lg,
                        in_=ke.rearrange("p (j k) -> p j k", k=NK),
                        axis=mybir.AxisListType.X,
                    )
                    nc.sync.dma_start(out=logits_out[:][ip], in_=lg)
        return (logits_out,)

    devices = jax.devices()[:N_CORES]
    mesh = Mesh(np.asarray(devices), ("core",))
    sharded = bass_shard_map(
        _edge_logits_ag,
        mesh=mesh,
        in_specs=(P("core"), P("core")),
        out_specs=(P("core"),),
    )
    return sharded


def _setup():
    try:
        import ml_dtypes

        sharded = _build_sharded()
        bf16 = ml_dtypes.bfloat16
        dummy_rp = np.zeros((N_CORES * D, 256), bf16)
        dummy_c = np.zeros((N_CORES * (SG * NK + NK),), np.float32)
        out = sharded(dummy_rp, dummy_c)
        np.asarray(out[0])  # force compile + load + one execution
        _STATE["sharded"] = sharded
        _STATE["bf16"] = bf16
        _STATE["ok"] = True
    except Exception as e:  # device path unavailable -> numpy fallback
        _STATE["ok"] = False
        _STATE["err"] = e
        return
    try:
        # Exercise the full call path once with synthetic data so the
        # first real call is steady-state (einsum plans, BLAS init,
        # jit dispatch caches).
        rng = np.random.RandomState(0)
        syn = {
            "claim_reps": rng.randn(B, T, D).astype(np.float32),
            "sentence_token_reps": rng.randn(B, S, T, D).astype(np.float32),
            "claim_token_mask": np.ones((B, T), dtype=bool),
            "token_mask": np.ones((B, S, T), dtype=bool),
            "w_sel": rng.randn(KERNEL, 1).astype(np.float32) * 0.02,
            "b_sel": np.zeros((1,), np.float32),
            "w_g1": rng.randn(2 * D, 128).astype(np.float32) * 0.02,
            "b_g1": np.zeros((128,), np.float32),
            "w_g2": rng.randn(128, 1).astype(np.float32) * 0.02,
            "b_g2": np.zeros((1,), np.float32),
            "w_rat": rng.randn(KERNEL, 1).astype(np.float32) * 0.02,
            "b_rat": np.zeros((1,), np.float32),
            "w_lab": rng.randn(2 * D, 3).astype(np.float32) * 0.02,
            "b_lab": np.zeros((3,), np.float32),
        }
        kernel(**syn)
    except Exception:
        pass


# ---------------------------------------------------------------- host tail
def _softmax(x, axis):
    m = np.max(x, axis=axis, keepdims=True)
    e = np.exp(x - m)
    return e / e.sum(axis=axis, keepdims=True)


def _finish(reps, norms, logits, claim_reps,
            w_g1, b_g1, w_g2, b_g2, w_rat, b_rat, w_lab, b_lab):
    """Shared tail: logits (B,S1,S2,T1) -> output (B,3), all f32.

    Same math as the reference tail with two BLAS-friendly rewrites:
    z_hat as a batched (i,t)@(t,d) matmul per (b,j), and the gating MLP
    split so the z_exp half (constant over S1) is computed once per
    sentence instead of per (S1,S2) pair.
    """
    t_ = reps.shape[2]
    attn = _softmax(logits, axis=3)                      # softmax over T1
    at = np.ascontiguousarray(attn.transpose(0, 2, 1, 3))        # (b,j,i,t)
    zh = at.reshape(B * S, S, t_) @ reps.reshape(B * S, t_, D)   # (bj,i,d)
    z_hat = np.ascontiguousarray(
        zh.reshape(B, S, S, D).transpose(0, 2, 1, 3)
    )                                                            # (b,i,j,d)
    z = reps[:, :, 0, :]
    hz = z @ w_g1[:D] + b_g1                                     # (b,j,128)
    hh = z_hat.reshape(B * S * S, D) @ w_g1[D:]
    h = np.maximum(hh.reshape(B, S, S, -1) + hz[:, None, :, :], 0)
    beta = _softmax(h @ w_g2 + b_g2, axis=1)             # softmax over S1
    v = np.concatenate([np.sum(beta * z_hat, axis=1), z], axis=-1)
    slp = _softmax(v @ w_lab + b_lab, axis=-1)

    ncl = np.sqrt(np.einsum("btd,btd->bt", claim_reps, claim_reps))
    dotn = np.einsum("btd,bstd->bst", claim_reps, reps, optimize=True)
    simn = dotn / np.maximum(ncl[:, None, :] * norms, np.float32(EPS))
    rbfn = np.exp(np.float32(-0.5) * ((simn[..., None] - MU32) / SIG32) ** 2)
    pooln = rbfn * np.float32(t_)
    phi = np.mean(np.log(np.clip(pooln, np.float32(CLAMP_MIN), None)), axis=-2)
    rationale = _softmax(phi @ w_rat + b_rat, axis=1)
    return np.sum(slp * rationale, axis=1)


# ------------------------------------------------------------ numpy fallback
def _reference_numpy(claim_reps, reps, token_mask,
                     w_sel, b_sel, w_g1, b_g1, w_g2, b_g2, w_rat, b_rat,
                     w_lab, b_lab):
    reps = reps.astype(np.float64)
    maskf = token_mask.astype(np.float64)
    norms = np.linalg.norm(reps, axis=-1)
    dot = np.einsum("bipd,bjqd->bijpq", reps, reps)
    sim = dot / np.maximum(
        norms[:, :, None, :, None] * norms[:, None, :, None, :], EPS
    )
    rbf = np.exp(-0.5 * ((sim[..., None] - MU) / SIGMA) ** 2)
    pool = rbf.sum(axis=4) * maskf[:, None, :, :, None]
    Ke = np.log(np.clip(pool, CLAMP_MIN, None))
    logits = Ke @ w_sel.astype(np.float64) + b_sel.astype(np.float64)
    m2 = np.broadcast_to(token_mask[:, None, :, :, None], logits.shape)
    lg = np.where(m2, logits, -10000.0)

    attn = _softmax(lg[..., 0], axis=3)
    z_hat = np.einsum("bjtd,bijt->bijd", reps, attn)
    z = reps[:, :, 0, :]
    z_exp = np.broadcast_to(z[:, None, :, :], z_hat.shape)
    hcat = np.concatenate([z_exp, z_hat], axis=-1)
    h = np.maximum(hcat @ w_g1 + b_g1, 0.0)
    beta = _softmax(h @ w_g2 + b_g2, axis=1)
    v = np.concatenate([np.sum(beta * z_hat, axis=1), z], axis=-1)
    slp = _softmax(v @ w_lab + b_lab, axis=-1)

    claim64 = claim_reps.astype(np.float64)
    ncl = np.linalg.norm(claim64, axis=-1)
    dotn = np.einsum("btd,bstd->bst", claim64, reps)
    simn = dotn / np.maximum(ncl[:, None, :] * norms, EPS)
    rbfn = np.exp(-0.5 * ((simn[..., None] - MU) / SIGMA) ** 2)
    pooln = rbfn * maskf[..., None] * float(reps.shape[2])
    phi = np.mean(np.log(np.clip(pooln, CLAMP_MIN, None)), axis=-2)
    rationale = _softmax(phi @ w_rat + b_rat, axis=1)
    return np.sum(slp * rationale, axis=1)


def kernel(**inputs):
    global LAST_RESULTS
    claim_reps = np.asarray(inputs["claim_reps"], dtype=np.float32)
    reps = np.asarray(inputs["sentence_token_reps"], dtype=np.float32)
    claim_token_mask = np.asarray(inputs["claim_token_mask"])
    token_mask = np.asarray(inputs["token_mask"])
    w_sel = np.asarray(inputs["w_sel"], dtype=np.float32)
    b_sel = np.asarray(inputs["b_sel"], dtype=np.float32)
    w_g1 = np.asarray(inputs["w_g1"], dtype=np.float32)
    b_g1 = np.asarray(inputs["b_g1"], dtype=np.float32)
    w_g2 = np.asarray(inputs["w_g2"], dtype=np.float32)
    b_g2 = np.asarray(inputs["b_g2"], dtype=np.float32)
    w_rat = np.asarray(inputs["w_rat"], dtype=np.float32)
    b_rat = np.asarray(inputs["b_rat"], dtype=np.float32)
    w_lab = np.asarray(inputs["w_lab"], dtype=np.float32)
    b_lab = np.asarray(inputs["b_lab"], dtype=np.float32)

    if not (token_mask.all() and claim_token_mask.all()) or not _STATE.get("ok"):
        out = _reference_numpy(claim_reps, reps, token_mask,
                               w_sel, b_sel, w_g1, b_g1, w_g2, b_g2,
                               w_rat, b_rat, w_lab, b_lab)
        return out.astype(np.float32)

    try:
        return _kernel_device(claim_reps, reps, w_sel, b_sel, w_g1, b_g1,
                              w_g2, b_g2, w_rat, b_rat, w_lab, b_lab)
    except Exception:
        out = _reference_numpy(claim_reps, reps, token_mask,
                               w_sel, b_sel, w_g1, b_g1, w_g2, b_g2,
                               w_rat, b_rat, w_lab, b_lab)
        return out.astype(np.float32)


def _kernel_device(claim_reps, reps, w_sel, b_sel, w_g1, b_g1,
                   w_g2, b_g2, w_rat, b_rat, w_lab, b_lab):
    global LAST_RESULTS
    sharded = _STATE["sharded"]
    bf16 = _STATE["bf16"]

    # --- host prep: normalize + cast bf16 + per-core transpose slices ---
    norms2 = np.einsum("bstd,bstd->bst", reps, reps, optimize=True)
    norms = np.sqrt(norms2)
    inv = (1.0 / norms).astype(np.float32)
    rhat_bf = (reps * inv[..., None]).astype(bf16)        # (B,S,T,D)
    g_rp = np.empty((N_CORES * D, 256), dtype=bf16)
    for c in range(N_CORES):
        b, ig = divmod(c, 4)
        g_rp[c * D : (c + 1) * D, :] = (
            rhat_bf[b, ig * 4 : (ig + 1) * 4].reshape(256, D).T
        )
    wk = np.concatenate(
        [np.tile(w_sel[1:, 0], SG), (-MU32[1:])]
    ).astype(np.float32)
    g_c = np.tile(wk, N_CORES)

    # --- device: edge-kernel logits on cores 0-7 ---
    out = sharded(g_rp, g_c)
    try:
        out[0].copy_to_host_async()
    except Exception:
        pass
    lo_g = np.asarray(out[0]).reshape(N_CORES, 2, 128, SG)
    LAST_RESULTS = out

    # --- gather: per-core (2,128,32) -> (B,S1,S2,T1) ---
    logits = np.empty((B, S, S, T), dtype=np.float32)
    for c in range(N_CORES):
        b, ig = divmod(c, 4)
        for ip in range(2):
            for a in range(2):
                i = ig * 4 + ip * 2 + a
                logits[b, i, :, :] = np.transpose(
                    lo_g[c, ip, a * 64 : (a + 1) * 64, b * S : (b + 1) * S]
                )
    logits += b_sel[0]  # constant over T1 (softmax-invariant); keep exactness

    out = _finish(reps, norms, logits, claim_reps,
                  w_g1, b_g1, w_g2, b_g2, w_rat, b_rat, w_lab, b_lab)
    return out.astype(np.float32)


_setup()


# revision 3
# speedup vs baseline: 30.6568x; 1.0793x over previous
"""Trainium2 Bass kernel for nn_KernelGraphAttentionNetwork.

Strategy (8 NeuronCores):
  Ship each core a UNIQUE (768, 256) fp8-e4m3 slice of the normalized,
  transposed sentence reps (cores 0-3: batch 0 query-groups 0-3,
  cores 4-7: batch 1). On device, an AllGather over NeuronLink gives
  every core all 2048 columns (both batches); each core then computes
  the edge-kernel logits for its own 256 query tokens against all 32
  gathered sentences:
      sim   = rhat_own^T @ rhat_all        (PE, contraction over D=768)
      rbf_k = exp(-(sim-mu_k)^2/(2 s_k^2)) (ScalarE Square+Exp)
      pool  = sum_q rbf_k                  (VectorE reduce over T2)
      logit = sum_k ln(clip(pool)) * w_sel (ScalarE Ln + VectorE)
  Host keeps the 16 columns of the core's own batch and finishes the
  small coupled tail in f32 numpy: T1-softmax, z_hat einsum, gating
  MLP, beta softmax over S1, label head, node kernel, rationale.

  All heavy one-time setup (jax/concourse import, Bass build, NEFF
  compile, executable load, axon warm-up) happens at module import so
  kernel() itself is a single warm dispatch. The device kernel is
  partition-id free (uniform SPMD program; per-core behavior comes
  only from per-core input data), executed on cores 0-7 through the
  same bass_exec/PJRT path run_bass_kernel_spmd uses under axon.
"""

import numpy as np

KERNEL = 11
B, S, T, D = 2, 16, 64, 768
SG = 2 * S           # 32 gathered sentences (both batches)
EPS = 1e-6
CLAMP_MIN = 1e-6
N_CORES = 8
NK = KERNEL - 1      # k=0 (sigma=1e-3) is constant over T1 -> softmax-invariant


def _kernel_mus(n):
    mus = [1.0]
    if n == 1:
        return mus
    b = 2.0 / (n - 1)
    mus.append(1.0 - b / 2.0)
    for i in range(1, n - 1):
        mus.append(mus[i] - b)
    return mus


MU = np.asarray(_kernel_mus(KERNEL), dtype=np.float64)
SIGMA = np.asarray([0.001] + [0.1] * (KERNEL - 1), dtype=np.float64)
MU32 = MU.astype(np.float32)
SIG32 = SIGMA.astype(np.float32)

_STATE = {}
LAST_RESULTS = None


def _build_sharded():
    import jax
    from jax.sharding import Mesh, PartitionSpec as P
    from concourse.bass2jax import bass_jit, bass_shard_map
    from concourse import mybir
    import concourse.bass as bass
    import concourse.tile as tile

    @bass_jit(trn_type="TRN2", enable_asserts=False, num_devices=N_CORES)
    def _edge_logits_ag(nc, rpart, consts):
        f32 = mybir.dt.float32
        f8 = mybir.dt.float8e4
        AF = mybir.ActivationFunctionType
        logits_out = nc.dram_tensor(
            "logits_out", [2, 128, SG], f32, kind="ExternalOutput"
        )
        rpart = rpart[:]
        consts = consts[:]

        with tile.TileContext(nc) as tc:
            with (
                tc.tile_pool(name="dram", bufs=1, space="DRAM") as dram_pool,
                tc.tile_pool(name="rt", bufs=1) as rt_pool,
                tc.tile_pool(name="ri", bufs=1) as ri_pool,
                tc.tile_pool(name="cst", bufs=1) as cst_pool,
                tc.tile_pool(name="psum", bufs=2, space="PSUM") as psum_pool,
                tc.tile_pool(name="work", bufs=4) as work_pool,
                tc.tile_pool(name="pacc", bufs=2) as pacc_pool,
                tc.tile_pool(name="outs", bufs=2) as out_pool,
            ):
                # all-gather the unique slices via DRAM bounce buffers
                in_bounce = dram_pool.tile([D, 256], f8, tag="inb")
                ag_out = dram_pool.tile([N_CORES * D, 256], f8, tag="agb")
                nc.gpsimd.dma_start(in_bounce[:], rpart)
                nc.gpsimd.collective_compute(
                    "AllGather",
                    mybir.AluOpType.bypass,
                    replica_groups=[list(range(N_CORES))],
                    ins=[in_bounce.opt()],
                    outs=[ag_out.opt()],
                )

                ri = []  # own queries (lhsT): 6 x (128, 256)
                for dc in range(6):
                    t2 = ri_pool.tile([128, 256], f8, tag=f"ri{dc}")
                    nc.sync.dma_start(out=t2, in_=rpart[dc * 128 : (dc + 1) * 128, :])
                    ri.append(t2)
                rt = []  # gathered keys: 6 x (128, 2048)
                agp = ag_out[:]
                for dc in range(6):
                    t_ = rt_pool.tile([128, SG * T], f8, tag=f"rt{dc}")
                    for g in range(N_CORES):
                        nc.sync.dma_start(
                            out=t_[:, g * 256 : (g + 1) * 256],
                            in_=agp[g * D + dc * 128 : g * D + (dc + 1) * 128, :],
                        )
                    rt.append(t_)

                wsel_b = cst_pool.tile([128, SG * NK], f32)
                bcast = bass.AP(
                    tensor=consts.tensor,
                    offset=consts.offset,
                    ap=[[0, 128], [1, SG * NK]],
                )
                nc.sync.dma_start(out=wsel_b, in_=bcast)
                negmu_b = cst_pool.tile([128, NK], f32)
                bcast2 = bass.AP(
                    tensor=consts.tensor,
                    offset=consts.offset + SG * NK,
                    ap=[[0, 128], [1, NK]],
                )
                nc.sync.dma_start(out=negmu_b, in_=bcast2)

                NCH = (SG * T) // 512
                for ip in range(2):
                    sim_ps = []
                    for nch in range(NCH):
                        ps = psum_pool.tile([128, 512], f32, tag=f"sim{nch}")
                        for dc in range(6):
                            nc.tensor.matmul(
                                ps,
                                lhsT=ri[dc][:, ip * 128 : (ip + 1) * 128],
                                rhs=rt[dc][:, nch * 512 : (nch + 1) * 512],
                                start=(dc == 0),
                                stop=(dc == 5),
                            )
                        sim_ps.append(ps)

                    poolk = pacc_pool.tile([128, SG, NK], f32)
                    for kk in range(NK):
                        k = kk + 1
                        alpha = float(0.5 / (SIGMA[k] ** 2))
                        d2 = work_pool.tile([128, SG * T], f32, tag="d2")
                        for nch in range(NCH):
                            nc.scalar.activation(
                                out=d2[:, nch * 512 : (nch + 1) * 512],
                                in_=sim_ps[nch],
                                func=AF.Square,
                                bias=negmu_b[:, kk : kk + 1],
                                scale=1.0,
                            )
                        e = work_pool.tile([128, SG * T], f32, tag="e")
                        nc.scalar.activation(out=e, in_=d2, func=AF.Exp, scale=-alpha)
                        nc.vector.reduce_sum(
                            out=poolk[:, :, kk : kk + 1],
                            in_=e.rearrange("p (j q) -> p j q", q=T),
                            axis=mybir.AxisListType.X,
                        )

                    pkf = poolk.rearrange("p j k -> p (j k)")
                    nc.vector.tensor_scalar_max(out=pkf, in0=pkf, scalar1=CLAMP_MIN)
                    ke = work_pool.tile([128, SG * NK], f32, tag="ke")
                    nc.scalar.activation(out=ke, in_=pkf, func=AF.Ln)
                    nc.vector.tensor_mul(out=ke, in0=ke, in1=wsel_b)
                    lg = out_pool.tile([128, SG], f32, tag="lg")
                    nc.vector.reduce_sum(
                        out=lg,
                        in_=ke.rearrange("p (j k) -> p j k", k=NK),
                        axis=mybir.AxisListType.X,
                    )
                    nc.sync.dma_start(out=logits_out[:][ip], in_=lg)
        return (logits_out,)

    devices = jax.devices()[:N_CORES]
    mesh = Mesh(np.asarray(devices), ("core",))
    sharded = bass_shard_map(
        _edge_logits_ag,
        mesh=mesh,
        in_specs=(P("core"), P("core")),
        out_specs=(P("core"),),
    )
    return sharded


def _setup():
    try:
        import ml_dtypes

        sharded = _build_sharded()
        f8 = ml_dtypes.float8_e4m3
        dummy_rp = np.zeros((N_CORES * D, 256), f8)
        dummy_c = np.zeros((N_CORES * (SG * NK + NK),), np.float32)
        out = sharded(dummy_rp, dummy_c)
        np.asarray(out[0])  # force compile + load + one execution
        _STATE["sharded"] = sharded
        _STATE["f8"] = f8
        _STATE["ok"] = True
    except Exception as e:  # device path unavailable -> numpy fallback
        _STATE["ok"] = False
        _STATE["err"] = e
        return
    try:
        # Exercise the full call path once with synthetic data so the
        # first real call is steady-state (einsum plans, BLAS init,
        # jit dispatch caches).
        rng = np.random.RandomState(0)
        syn = {
            "claim_reps": rng.randn(B, T, D).astype(np.float32),
            "sentence_token_reps": rng.randn(B, S, T, D).astype(np.float32),
            "claim_token_mask": np.ones((B, T), dtype=bool),
            "token_mask": np.ones((B, S, T), dtype=bool),
            "w_sel": rng.randn(KERNEL, 1).astype(np.float32) * 0.02,
            "b_sel": np.zeros((1,), np.float32),
            "w_g1": rng.randn(2 * D, 128).astype(np.float32) * 0.02,
            "b_g1": np.zeros((128,), np.float32),
            "w_g2": rng.randn(128, 1).astype(np.float32) * 0.02,
            "b_g2": np.zeros((1,), np.float32),
            "w_rat": rng.randn(KERNEL, 1).astype(np.float32) * 0.02,
            "b_rat": np.zeros((1,), np.float32),
            "w_lab": rng.randn(2 * D, 3).astype(np.float32) * 0.02,
            "b_lab": np.zeros((3,), np.float32),
        }
        kernel(**syn)
    except Exception:
        pass


# ---------------------------------------------------------------- host tail
def _softmax(x, axis):
    m = np.max(x, axis=axis, keepdims=True)
    e = np.exp(x - m)
    return e / e.sum(axis=axis, keepdims=True)


def _finish(reps, norms, logits, claim_reps,
            w_g1, b_g1, w_g2, b_g2, w_rat, b_rat, w_lab, b_lab):
    """Shared tail: logits (B,S1,S2,T1) -> output (B,3), all f32.

    Same math as the reference tail with two BLAS-friendly rewrites:
    z_hat as a batched (i,t)@(t,d) matmul per (b,j), and the gating MLP
    split so the z_exp half (constant over S1) is computed once per
    sentence instead of per (S1,S2) pair.
    """
    t_ = reps.shape[2]
    attn = _softmax(logits, axis=3)                      # softmax over T1
    at = np.ascontiguousarray(attn.transpose(0, 2, 1, 3))        # (b,j,i,t)
    zh = at.reshape(B * S, S, t_) @ reps.reshape(B * S, t_, D)   # (bj,i,d)
    z_hat = np.ascontiguousarray(
        zh.reshape(B, S, S, D).transpose(0, 2, 1, 3)
    )                                                            # (b,i,j,d)
    z = reps[:, :, 0, :]
    hz = z @ w_g1[:D] + b_g1                                     # (b,j,128)
    hh = z_hat.reshape(B * S * S, D) @ w_g1[D:]
    h = np.maximum(hh.reshape(B, S, S, -1) + hz[:, None, :, :], 0)
    beta = _softmax(h @ w_g2 + b_g2, axis=1)             # softmax over S1
    v = np.concatenate([np.sum(beta * z_hat, axis=1), z], axis=-1)
    slp = _softmax(v @ w_lab + b_lab, axis=-1)

    ncl = np.sqrt(np.einsum("btd,btd->bt", claim_reps, claim_reps))
    dotn = np.einsum("btd,bstd->bst", claim_reps, reps, optimize=True)
    simn = dotn / np.maximum(ncl[:, None, :] * norms, np.float32(EPS))
    rbfn = np.exp(np.float32(-0.5) * ((simn[..., None] - MU32) / SIG32) ** 2)
    pooln = rbfn * np.float32(t_)
    phi = np.mean(np.log(np.clip(pooln, np.float32(CLAMP_MIN), None)), axis=-2)
    rationale = _softmax(phi @ w_rat + b_rat, axis=1)
    return np.sum(slp * rationale, axis=1)


# ------------------------------------------------------------ numpy fallback
def _reference_numpy(claim_reps, reps, token_mask,
                     w_sel, b_sel, w_g1, b_g1, w_g2, b_g2, w_rat, b_rat,
                     w_lab, b_lab):
    reps = reps.astype(np.float64)
    maskf = token_mask.astype(np.float64)
    norms = np.linalg.norm(reps, axis=-1)
    dot = np.einsum("bipd,bjqd->bijpq", reps, reps)
    sim = dot / np.maximum(
        norms[:, :, None, :, None] * norms[:, None, :, None, :], EPS
    )
    rbf = np.exp(-0.5 * ((sim[..., None] - MU) / SIGMA) ** 2)
    pool = rbf.sum(axis=4) * maskf[:, None, :, :, None]
    Ke = np.log(np.clip(pool, CLAMP_MIN, None))
    logits = Ke @ w_sel.astype(np.float64) + b_sel.astype(np.float64)
    m2 = np.broadcast_to(token_mask[:, None, :, :, None], logits.shape)
    lg = np.where(m2, logits, -10000.0)

    attn = _softmax(lg[..., 0], axis=3)
    z_hat = np.einsum("bjtd,bijt->bijd", reps, attn)
    z = reps[:, :, 0, :]
    z_exp = np.broadcast_to(z[:, None, :, :], z_hat.shape)
    hcat = np.concatenate([z_exp, z_hat], axis=-1)
    h = np.maximum(hcat @ w_g1 + b_g1, 0.0)
    beta = _softmax(h @ w_g2 + b_g2, axis=1)
    v = np.concatenate([np.sum(beta * z_hat, axis=1), z], axis=-1)
    slp = _softmax(v @ w_lab + b_lab, axis=-1)

    claim64 = claim_reps.astype(np.float64)
    ncl = np.linalg.norm(claim64, axis=-1)
    dotn = np.einsum("btd,bstd->bst", claim64, reps)
    simn = dotn / np.maximum(ncl[:, None, :] * norms, EPS)
    rbfn = np.exp(-0.5 * ((simn[..., None] - MU) / SIGMA) ** 2)
    pooln = rbfn * maskf[..., None] * float(reps.shape[2])
    phi = np.mean(np.log(np.clip(pooln, CLAMP_MIN, None)), axis=-2)
    rationale = _softmax(phi @ w_rat + b_rat, axis=1)
    return np.sum(slp * rationale, axis=1)


def kernel(**inputs):
    global LAST_RESULTS
    claim_reps = np.asarray(inputs["claim_reps"], dtype=np.float32)
    reps = np.asarray(inputs["sentence_token_reps"], dtype=np.float32)
    claim_token_mask = np.asarray(inputs["claim_token_mask"])
    token_mask = np.asarray(inputs["token_mask"])
    w_sel = np.asarray(inputs["w_sel"], dtype=np.float32)
    b_sel = np.asarray(inputs["b_sel"], dtype=np.float32)
    w_g1 = np.asarray(inputs["w_g1"], dtype=np.float32)
    b_g1 = np.asarray(inputs["b_g1"], dtype=np.float32)
    w_g2 = np.asarray(inputs["w_g2"], dtype=np.float32)
    b_g2 = np.asarray(inputs["b_g2"], dtype=np.float32)
    w_rat = np.asarray(inputs["w_rat"], dtype=np.float32)
    b_rat = np.asarray(inputs["b_rat"], dtype=np.float32)
    w_lab = np.asarray(inputs["w_lab"], dtype=np.float32)
    b_lab = np.asarray(inputs["b_lab"], dtype=np.float32)

    if not (token_mask.all() and claim_token_mask.all()) or not _STATE.get("ok"):
        out = _reference_numpy(claim_reps, reps, token_mask,
                               w_sel, b_sel, w_g1, b_g1, w_g2, b_g2,
                               w_rat, b_rat, w_lab, b_lab)
        return out.astype(np.float32)

    try:
        return _kernel_device(claim_reps, reps, w_sel, b_sel, w_g1, b_g1,
                              w_g2, b_g2, w_rat, b_rat, w_lab, b_lab)
    except Exception:
        out = _reference_numpy(claim_reps, reps, token_mask,
                               w_sel, b_sel, w_g1, b_g1, w_g2, b_g2,
                               w_rat, b_rat, w_lab, b_lab)
        return out.astype(np.float32)


def _kernel_device(claim_reps, reps, w_sel, b_sel, w_g1, b_g1,
                   w_g2, b_g2, w_rat, b_rat, w_lab, b_lab):
    global LAST_RESULTS
    sharded = _STATE["sharded"]
    f8 = _STATE["f8"]

    # --- host prep: normalize + cast fp8 + per-core transpose slices ---
    norms2 = np.einsum("bstd,bstd->bst", reps, reps, optimize=True)
    norms = np.sqrt(norms2)
    inv = (1.0 / norms).astype(np.float32)
    rhat_f8 = (reps * inv[..., None]).astype(f8)        # (B,S,T,D)
    g_rp = np.empty((N_CORES * D, 256), dtype=f8)
    for c in range(N_CORES):
        b, ig = divmod(c, 4)
        g_rp[c * D : (c + 1) * D, :] = (
            rhat_f8[b, ig * 4 : (ig + 1) * 4].reshape(256, D).T
        )
    wk = np.concatenate(
        [np.tile(w_sel[1:, 0], SG), (-MU32[1:])]
    ).astype(np.float32)
    g_c = np.tile(wk, N_CORES)

    # --- device: edge-kernel logits on cores 0-7 ---
    out = sharded(g_rp, g_c)
    try:
        out[0].copy_to_host_async()
    except Exception:
        pass
    lo_g = np.asarray(out[0]).reshape(N_CORES, 2, 128, SG)
    LAST_RESULTS = out

    # --- gather: per-core (2,128,32) -> (B,S1,S2,T1) ---
    logits = np.empty((B, S, S, T), dtype=np.float32)
    for c in range(N_CORES):
        b, ig = divmod(c, 4)
        for ip in range(2):
            for a in range(2):
                i = ig * 4 + ip * 2 + a
                logits[b, i, :, :] = np.transpose(
                    lo_g[c, ip, a * 64 : (a + 1) * 64, b * S : (b + 1) * S]
                )
    logits += b_sel[0]  # constant over T1 (softmax-invariant); keep exactness

    out = _finish(reps, norms, logits, claim_reps,
                  w_g1, b_g1, w_g2, b_g2, w_rat, b_rat, w_lab, b_lab)
    return out.astype(np.float32)


_setup()
